# revision 2
# baseline (speedup 1.0000x reference)
"""Devign model (GGNN + conv readout) Trainium2 kernel.

Data-parallel over the batch dim: 64 graphs -> 8 NeuronCores x 8 graphs.
Everything on-device runs in bf16 matmuls with fp32 PSUM accumulation, in a
feature-major layout ([feature, node] on SBUF partitions) so no transposes are
needed anywhere. The GGNN scatter-add aggregation is reformulated as dense
matmuls against per-graph adjacency-count matrices A^T[(type,src), dst] built
on the host, with an extra in-degree chunk that applies b_lin exactly.
"""

import numpy as np
import ml_dtypes

import concourse.bass as bass
import concourse.bacc as bacc
import concourse.mybir as mybir
import concourse.tile as tile
from concourse.bass_utils import run_bass_kernel_spmd

bf16 = ml_dtypes.bfloat16
FP32 = mybir.dt.float32
BF16 = mybir.dt.bfloat16

# Problem constants (hardcoded per the spec).
B, N, DIN, D, T, NUM_STEPS = 64, 510, 256, 256, 4, 6
NPAD = 512          # padded nodes per graph
GPC = 8             # graphs per core
N_CORES = 8
P = 128
NCHUNK = 17         # 16 adjacency chunks (4 types x 4 src tiles) + 1 indeg chunk
L1 = 508            # conv1 output length (510 - 3 + 1)
LP1 = 253           # after pool(3,2)
L2Y = 253           # conv2 (k=1) output length
L2Z = 252           # convc2 (k=2) output length
LF = 126            # after pool(2,2)

AF = mybir.ActivationFunctionType
ALU = mybir.AluOpType

_NC_CACHE = {}


def _build_nc():
    nc = bacc.Bacc("TRN2", target_bir_lowering=False, debug=False,
                   num_devices=N_CORES)

    # ---- DRAM parameters (per-core shapes) ----
    hT0_d = nc.declare_dram_parameter("hT0", [P, 2, GPC * NPAD], BF16, isOutput=False)
    AT_d = nc.declare_dram_parameter("AT", [GPC, NCHUNK, P, NPAD], BF16, isOutput=False)
    wcat_d = nc.declare_dram_parameter("Wcat", [P, 2, T * D], BF16, isOutput=False)
    bl_d = nc.declare_dram_parameter("BL", [T, D], BF16, isOutput=False)
    wih_d = nc.declare_dram_parameter("WihT", [P, 2, 3 * D], BF16, isOutput=False)
    whh_d = nc.declare_dram_parameter("WhhT", [P, 2, 3 * D], BF16, isOutput=False)
    brz_d = nc.declare_dram_parameter("brz", [P, 4], FP32, isOutput=False)
    bihn_d = nc.declare_dram_parameter("bihn", [P, 2], FP32, isOutput=False)
    bhhn_d = nc.declare_dram_parameter("bhhn", [P, 2], FP32, isOutput=False)
    w1_d = nc.declare_dram_parameter("W1T", [P, 6, D], BF16, isOutput=False)
    w2_d = nc.declare_dram_parameter("W2T", [P, 2, D], BF16, isOutput=False)
    wc1_d = nc.declare_dram_parameter("Wc1T", [P, 12, 2 * D], BF16, isOutput=False)
    wc2_d = nc.declare_dram_parameter("Wc2T", [P, 8, 2 * D], BF16, isOutput=False)
    cb1_d = nc.declare_dram_parameter("cb1", [P, 2], FP32, isOutput=False)
    cb2_d = nc.declare_dram_parameter("cb2", [P, 2], FP32, isOutput=False)
    cc1_d = nc.declare_dram_parameter("cc1", [P, 4], FP32, isOutput=False)
    cc2_d = nc.declare_dram_parameter("cc2", [P, 4], FP32, isOutput=False)
    wy_d = nc.declare_dram_parameter("wyT", [P, 2, 1], BF16, isOutput=False)
    wz_d = nc.declare_dram_parameter("wzT", [P, 4, 1], BF16, isOutput=False)
    mlpb_d = nc.declare_dram_parameter("mlpb", [1, 2], FP32, isOutput=False)
    out_d = nc.declare_dram_parameter("out", [1, GPC], FP32, isOutput=True)

    with tile.TileContext(nc) as tc:
        with (
            tc.tile_pool(name="const", bufs=1) as cst,
            tc.tile_pool(name="state", bufs=1) as st,
            tc.tile_pool(name="atp", bufs=8) as atp,
            tc.tile_pool(name="mp", bufs=8) as mp,
            tc.tile_pool(name="rzp", bufs=8) as rzp,
            tc.tile_pool(name="gp", bufs=3) as gp,
            tc.tile_pool(name="cvp", bufs=4) as cvp,
            tc.tile_pool(name="psm", bufs=2, space="PSUM") as psm,
            tc.tile_pool(name="psa", bufs=2, space="PSUM") as psa,
            tc.tile_pool(name="psg", bufs=4, space="PSUM") as psg,
        ):
            # ---- load constants ----
            wcat = cst.tile([P, 2, T * D], BF16)
            nc.sync.dma_start(wcat[:], wcat_d[:])
            bl = cst.tile([T, D], BF16)
            nc.sync.dma_start(bl[:], bl_d[:])
            wih = cst.tile([P, 2, 3 * D], BF16)
            nc.sync.dma_start(wih[:], wih_d[:])
            whh = cst.tile([P, 2, 3 * D], BF16)
            nc.sync.dma_start(whh[:], whh_d[:])
            brz = cst.tile([P, 4], FP32)
            nc.sync.dma_start(brz[:], brz_d[:])
            bihn = cst.tile([P, 2], FP32)
            nc.sync.dma_start(bihn[:], bihn_d[:])
            bhhn = cst.tile([P, 2], FP32)
            nc.sync.dma_start(bhhn[:], bhhn_d[:])
            w1 = cst.tile([P, 6, D], BF16)
            nc.sync.dma_start(w1[:], w1_d[:])
            w2 = cst.tile([P, 2, D], BF16)
            nc.sync.dma_start(w2[:], w2_d[:])
            wc1 = cst.tile([P, 12, 2 * D], BF16)
            nc.sync.dma_start(wc1[:], wc1_d[:])
            wc2 = cst.tile([P, 8, 2 * D], BF16)
            nc.sync.dma_start(wc2[:], wc2_d[:])
            cb1 = cst.tile([P, 2], FP32)
            nc.sync.dma_start(cb1[:], cb1_d[:])
            cb2 = cst.tile([P, 2], FP32)
            nc.sync.dma_start(cb2[:], cb2_d[:])
            cc1 = cst.tile([P, 4], FP32)
            nc.sync.dma_start(cc1[:], cc1_d[:])
            cc2 = cst.tile([P, 4], FP32)
            nc.sync.dma_start(cc2[:], cc2_d[:])
            wy = cst.tile([P, 2, 1], BF16)
            nc.sync.dma_start(wy[:], wy_d[:])
            wz = cst.tile([P, 4, 1], BF16)
            nc.sync.dma_start(wz[:], wz_d[:])
            mlpb = cst.tile([1, 2], FP32)
            nc.sync.dma_start(mlpb[:], mlpb_d[:])

            # ---- per-graph state tiles (feature-major [feat_part, kt, node]) ----
            feats = []
            hA = []
            hB = []
            aTs = []
            for g in range(GPC):
                f = st.tile([P, 2, NPAD], BF16, tag=f"feat{g}")
                nc.sync.dma_start(f[:], hT0_d[:, :, g * NPAD:(g + 1) * NPAD])
                feats.append(f)
                h0 = st.tile([P, 2, NPAD], BF16, tag=f"hA{g}")
                nc.sync.dma_start(h0[:], hT0_d[:, :, g * NPAD:(g + 1) * NPAD])
                hA.append(h0)
                h1 = st.tile([P, 2, NPAD], BF16, tag=f"hB{g}")
                hB.append(h1)
                a = st.tile([P, 2, NPAD], BF16, tag=f"aT{g}")
                nc.vector.memset(a[:], 0.0)
                aTs.append(a)

            # ================= GGNN steps =================
            for step in range(NUM_STEPS):
                hcur = hA if step % 2 == 0 else hB
                hnxt = hB if step % 2 == 0 else hA
                for g in range(GPC):
                    hg = hcur[g]
                    ag = aTs[g]
                    # --- m = h_g @ Wcat  (node-major [node, T*D]) ---
                    m_tiles = []
                    for i in range(4):
                        msb = mp.tile([P, T * D], BF16, tag="m")
                        for nt in range(2):
                            pm = psm.tile([P, 512], FP32, tag="pm")
                            for kt in range(2):
                                nc.tensor.matmul(
                                    pm[:],
                                    lhsT=hg[:, kt, i * P:(i + 1) * P],
                                    rhs=wcat[:, kt, nt * 512:(nt + 1) * 512],
                                    start=(kt == 0), stop=(kt == 1),
                                )
                            nc.vector.tensor_copy(msb[:, nt * 512:(nt + 1) * 512], pm[:])
                        m_tiles.append(msb)

                    # --- aggregation: aT_g = m_stack^T @ A^T_g (+ b_lin via indeg) ---
                    at_tiles = []
                    for c in range(NCHUNK):
                        atile = atp.tile([P, NPAD], BF16, tag="at")
                        nc.sync.dma_start(atile[:], AT_d[g, c])
                        at_tiles.append(atile)
                    pas = []
                    for mt in range(2):
                        pa_t = psa.tile([P, 512], FP32, tag="pa", name=f"pa{mt}")
                        pas.append(pa_t)
                    for c in range(NCHUNK):
                        for mt in range(2):
                            if c < 16:
                                t, i = c // 4, c % 4
                                nc.tensor.matmul(
                                    pas[mt][:],
                                    lhsT=m_tiles[i][:, t * D + mt * P: t * D + (mt + 1) * P],
                                    rhs=at_tiles[c][:],
                                    start=(c == 0), stop=False,
                                )
                            else:
                                nc.tensor.matmul(
                                    pas[mt][:],
                                    lhsT=bl[:, mt * P:(mt + 1) * P],
                                    rhs=at_tiles[c][:T, :],
                                    start=False, stop=True,
                                )
                    for mt in range(2):
                        nc.scalar.activation(ag[:, mt, :], pas[mt][:], AF.Identity)

                    # --- GRU ---
                    rz_tiles = []
                    for mt in range(4):
                        pr = psg.tile([P, 512], FP32, tag="pg")
                        for kt in range(2):
                            nc.tensor.matmul(
                                pr[:], lhsT=wih[:, kt, mt * P:(mt + 1) * P],
                                rhs=ag[:, kt, :], start=(kt == 0), stop=False)
                        for kt in range(2):
                            nc.tensor.matmul(
                                pr[:], lhsT=whh[:, kt, mt * P:(mt + 1) * P],
                                rhs=hg[:, kt, :], start=False, stop=(kt == 1))
                        rzt = rzp.tile([P, 512], BF16, tag="rz")
                        nc.scalar.activation(rzt[:], pr[:], AF.Sigmoid,
                                             bias=brz[:, mt:mt + 1])
                        rz_tiles.append(rzt)
                    for mt in range(2):
                        pi = psg.tile([P, 512], FP32, tag="pg")
                        for kt in range(2):
                            nc.tensor.matmul(
                                pi[:], lhsT=wih[:, kt, 2 * D + mt * P: 2 * D + (mt + 1) * P],
                                rhs=ag[:, kt, :], start=(kt == 0), stop=(kt == 1))
                        ph = psg.tile([P, 512], FP32, tag="pg")
                        for kt in range(2):
                            nc.tensor.matmul(
                                ph[:], lhsT=whh[:, kt, 2 * D + mt * P: 2 * D + (mt + 1) * P],
                                rhs=hg[:, kt, :], start=(kt == 0), stop=(kt == 1))
                        innp = gp.tile([P, 512], BF16, tag="innp")
                        nc.scalar.activation(innp[:], pi[:], AF.Identity,
                                             bias=bihn[:, mt:mt + 1])
                        hnp = gp.tile([P, 512], BF16, tag="hnp")
                        nc.scalar.activation(hnp[:], ph[:], AF.Identity,
                                             bias=bhhn[:, mt:mt + 1])
                        t1 = gp.tile([P, 512], BF16, tag="t1")
                        nc.vector.tensor_mul(t1[:], rz_tiles[mt][:], hnp[:])
                        nc.vector.tensor_add(t1[:], innp[:], t1[:])
                        nsb = gp.tile([P, 512], BF16, tag="nsb")
                        nc.scalar.activation(nsb[:], t1[:], AF.Tanh)
                        dsb = gp.tile([P, 512], BF16, tag="dsb")
                        nc.vector.tensor_tensor(dsb[:], hg[:, mt, :], nsb[:],
                                                op=ALU.subtract)
                        nc.vector.tensor_mul(dsb[:], rz_tiles[2 + mt][:], dsb[:])
                        nc.vector.tensor_add(hnxt[g][:, mt, :], nsb[:], dsb[:])

            hfin = hA if NUM_STEPS % 2 == 0 else hB

            # ================= conv readout =================
            res = cst.tile([1, GPC], FP32)
            for g in range(GPC):
                hg = hfin[g]
                fg = feats[g]
                # --- Y branch ---
                y1p = []
                for mt in range(2):
                    pm = psm.tile([P, 512], FP32, tag="pm")
                    first = True
                    for k in range(3):
                        for kt in range(2):
                            nc.tensor.matmul(
                                pm[:, :L1],
                                lhsT=w1[:, k * 2 + kt, mt * P:(mt + 1) * P],
                                rhs=hg[:, kt, k:k + L1],
                                start=first, stop=(k == 2 and kt == 1))
                            first = False
                    y1 = cvp.tile([P, 512], BF16, tag="y1")
                    nc.scalar.activation(y1[:, :L1], pm[:, :L1], AF.Relu,
                                         bias=cb1[:, mt:mt + 1])
                    yp = cvp.tile([P, LP1], BF16, tag="y1p")
                    nc.vector.tensor_tensor(yp[:], y1[:, 0:505:2], y1[:, 1:506:2],
                                            op=ALU.max)
                    nc.vector.tensor_tensor(yp[:], yp[:], y1[:, 2:507:2],
                                            op=ALU.max)
                    y1p.append(yp)
                y2p = []
                for mt in range(2):
                    pm = psm.tile([P, 512], FP32, tag="pm")
                    for kt in range(2):
                        nc.tensor.matmul(
                            pm[:, :L2Y],
                            lhsT=w2[:, kt, mt * P:(mt + 1) * P],
                            rhs=y1p[kt][:],
                            start=(kt == 0), stop=(kt == 1))
                    y2 = cvp.tile([P, L2Y], BF16, tag="y2")
                    nc.scalar.activation(y2[:], pm[:, :L2Y], AF.Relu,
                                         bias=cb2[:, mt:mt + 1])
                    yp = cvp.tile([P, LF], BF16, tag="y2p")
                    nc.vector.tensor_tensor(yp[:], y2[:, 0:251:2], y2[:, 1:252:2],
                                            op=ALU.max)
                    y2p.append(yp)
                pv = psa.tile([P, 512], FP32, tag="pa")
                for kt in range(2):
                    nc.tensor.matmul(pv[0:1, :LF], lhsT=wy[:, kt, :],
                                     rhs=y2p[kt][:], start=(kt == 0), stop=(kt == 1))
                ysb = cvp.tile([1, LF], FP32, tag="ysb")
                nc.scalar.activation(ysb[:], pv[0:1, :LF], AF.Identity,
                                     bias=mlpb[:, 0:1])

                # --- Z branch (channels = [h; feat]) ---
                z1p = []
                for mt in range(4):
                    pm = psm.tile([P, 512], FP32, tag="pm")
                    first = True
                    for k in range(3):
                        for kt in range(4):
                            src = hg if kt < 2 else fg
                            nc.tensor.matmul(
                                pm[:, :L1],
                                lhsT=wc1[:, k * 4 + kt, mt * P:(mt + 1) * P],
                                rhs=src[:, kt % 2, k:k + L1],
                                start=first, stop=(k == 2 and kt == 3))
                            first = False
                    z1 = cvp.tile([P, 512], BF16, tag="z1")
                    nc.scalar.activation(z1[:, :L1], pm[:, :L1], AF.Relu,
                                         bias=cc1[:, mt:mt + 1])
                    zp = cvp.tile([P, LP1], BF16, tag="z1p")
                    nc.vector.tensor_tensor(zp[:], z1[:, 0:505:2], z1[:, 1:506:2],
                                            op=ALU.max)
                    nc.vector.tensor_tensor(zp[:], zp[:], z1[:, 2:507:2],
                                            op=ALU.max)
                    z1p.append(zp)
                z2p = []
                for mt in range(4):
                    pm = psm.tile([P, 512], FP32, tag="pm")
                    first = True
                    for k in range(2):
                        for kt in range(4):
                            nc.tensor.matmul(
                                pm[:, :L2Z],
                                lhsT=wc2[:, k * 4 + kt, mt * P:(mt + 1) * P],
                                rhs=z1p[kt][:, k:k + L2Z],
                                start=first, stop=(k == 1 and kt == 3))
                            first = False
                    z2 = cvp.tile([P, L2Z], BF16, tag="z2")
                    nc.scalar.activation(z2[:], pm[:, :L2Z], AF.Relu,
                                         bias=cc2[:, mt:mt + 1])
                    zp = cvp.tile([P, LF], BF16, tag="z2p")
                    nc.vector.tensor_tensor(zp[:], z2[:, 0:251:2], z2[:, 1:252:2],
                                            op=ALU.max)
                    z2p.append(zp)
                pv = psa.tile([P, 512], FP32, tag="pa")
                for kt in range(4):
                    nc.tensor.matmul(pv[0:1, :LF], lhsT=wz[:, kt, :],
                                     rhs=z2p[kt][:], start=(kt == 0), stop=(kt == 3))
                zsb = cvp.tile([1, LF], FP32, tag="zsb")
                nc.scalar.activation(zsb[:], pv[0:1, :LF], AF.Identity,
                                     bias=mlpb[:, 1:2])

                # --- final: sigmoid(mean(y*z)) ---
                prod = cvp.tile([1, LF], FP32, tag="prod")
                nc.vector.tensor_mul(prod[:], ysb[:], zsb[:])
                ssum = cvp.tile([1, 1], FP32, tag="ssum")
                nc.vector.reduce_sum(ssum[:], prod[:], axis=mybir.AxisListType.X)
                nc.scalar.activation(res[:, g:g + 1], ssum[:], AF.Sigmoid,
                                     scale=1.0 / LF)

            nc.sync.dma_start(out_d[:], res[:])

    nc.compile()
    return nc


def _split_part(a, ntile):
    """[ntile*128, F...] -> [128, ntile, F...] with [p, t, ...] = a[t*128+p, ...]."""
    return np.ascontiguousarray(
        a.reshape(ntile, P, *a.shape[1:]).transpose(1, 0, *range(2, a.ndim + 1)))


def _prep_inputs(inputs):
    feat = np.asarray(inputs["feat"], np.float32)
    esrc = np.asarray(inputs["edge_src"]).astype(np.int64)
    edst = np.asarray(inputs["edge_dst"]).astype(np.int64)
    etyp = np.asarray(inputs["edge_type"]).astype(np.int64)

    # feature-major padded feat^T: per graph [256, 512]
    ftp = np.zeros((B, DIN, NPAD), np.float32)
    ftp[:, :, :N] = feat.transpose(0, 2, 1)

    # adjacency chunks: AT_all[g, c, s, d]; c = t*4 + src//128 for c<16, c=16 indeg
    AT_all = np.zeros((B, NCHUNK, P, NPAD), np.float32)
    g_of = esrc // N
    s_l = esrc % N
    d_l = edst % N
    np.add.at(AT_all, (g_of, etyp * 4 + s_l // P, s_l % P, d_l), 1.0)
    np.add.at(AT_all, (g_of, 16, etyp, d_l), 1.0)

    W_lin = np.asarray(inputs["W_lin"], np.float32)
    Wcat = W_lin.transpose(2, 0, 1).reshape(D, T * D)
    W_ih = np.asarray(inputs["W_ih"], np.float32)
    W_hh = np.asarray(inputs["W_hh"], np.float32)
    b_ih = np.asarray(inputs["b_ih"], np.float32)
    b_hh = np.asarray(inputs["b_hh"], np.float32)

    def convT(w):  # [O, I, K] -> [128, K*ktiles, O]
        O, I, K = w.shape
        kt = I // P
        arr = w.transpose(2, 1, 0).reshape(K, kt, P, O).transpose(2, 0, 1, 3)
        return np.ascontiguousarray(arr.reshape(P, K * kt, O))

    common = {
        "Wcat": _split_part(Wcat, 2).astype(bf16),
        "BL": np.asarray(inputs["b_lin"], np.float32).astype(bf16),
        "WihT": _split_part(np.ascontiguousarray(W_ih.T), 2).astype(bf16),
        "WhhT": _split_part(np.ascontiguousarray(W_hh.T), 2).astype(bf16),
        "brz": np.ascontiguousarray((b_ih + b_hh)[:2 * D].reshape(4, P).T),
        "bihn": np.ascontiguousarray(b_ih[2 * D:].reshape(2, P).T),
        "bhhn": np.ascontiguousarray(b_hh[2 * D:].reshape(2, P).T),
        "W1T": convT(np.asarray(inputs["conv1_w"], np.float32)).astype(bf16),
        "W2T": convT(np.asarray(inputs["conv2_w"], np.float32)).astype(bf16),
        "Wc1T": convT(np.asarray(inputs["convc1_w"], np.float32)).astype(bf16),
        "Wc2T": convT(np.asarray(inputs["convc2_w"], np.float32)).astype(bf16),
        "cb1": np.ascontiguousarray(np.asarray(inputs["conv1_b"], np.float32).reshape(2, P).T),
        "cb2": np.ascontiguousarray(np.asarray(inputs["conv2_b"], np.float32).reshape(2, P).T),
        "cc1": np.ascontiguousarray(np.asarray(inputs["convc1_b"], np.float32).reshape(4, P).T),
        "cc2": np.ascontiguousarray(np.asarray(inputs["convc2_b"], np.float32).reshape(4, P).T),
        "wyT": _split_part(np.ascontiguousarray(np.asarray(inputs["mlp_y_w"], np.float32).T), 2).astype(bf16),
        "wzT": _split_part(np.ascontiguousarray(np.asarray(inputs["mlp_z_w"], np.float32).T), 4).astype(bf16),
        "mlpb": np.array([[float(np.asarray(inputs["mlp_y_b"])[0]),
                           float(np.asarray(inputs["mlp_z_b"])[0])]], np.float32),
    }

    in_maps = []
    for c in range(N_CORES):
        sl = slice(c * GPC, (c + 1) * GPC)
        hT0 = ftp[sl].transpose(1, 0, 2).reshape(DIN, GPC * NPAD)
        m = dict(common)
        m["hT0"] = _split_part(hT0, 2).astype(bf16)
        m["AT"] = AT_all[sl].astype(bf16)
        in_maps.append(m)
    return in_maps


def kernel(**inputs):
    if "nc" not in _NC_CACHE:
        _NC_CACHE["nc"] = _build_nc()
    nc = _NC_CACHE["nc"]
    in_maps = _prep_inputs(inputs)
    res = run_bass_kernel_spmd(nc, in_maps, list(range(N_CORES)))
    return np.concatenate([res.results[c]["out"][0] for c in range(N_CORES)])
